# revision 18
# baseline (speedup 1.0000x reference)
"""Block-sparse attention kernel for Trainium2 (8 NeuronCores, SPMD).

Strategy (v2)
-------------
* Shard batch*heads (2*16 = 32 pairs) across 8 cores, 4 heads per core.
* Flash-style attention in S^T layout: S^T[k, q] via matmul(lhsT=K^T
  chunk, rhs=Q^T), with q/k inputs pre-transposed on the host in fp16
  and replicated into both partition halves so chunk pairs run as
  concurrent row-tiled K=64 matmuls (PE row groups 0-1 / 2-3).
* exp(sm_scale * S^T) is split across engines: the ACT (scalar) engine
  computes exact exp for a share of the chunks; the rest go through a
  two-instruction approximate path on the DVE - tensor_scalar computes
  the Schraudolph bit-trick i32(A*s + B) (bits of 2^i*(1+f)), and a
  custom DVE op applies a quadratic correction using h = w*bitcast(~w),
  a pure function of the mantissa fraction (max rel err ~0.52%).
  (Pool/GPSIMD cannot read PSUM, so it only gets masks/memsets.)
* The PV matmul contracts the full 128-row chunk (K=128 uses every PE
  row - splitting it into row tiles would just stream each output
  column twice), accumulating O^T + softmax denominators per pass.
* Q-pass granularity is 512 (PSUM: 4 score banks + 2 O banks +
  2 transpose banks = 8). End-phase (transpose O^T -> O on the tensor
  engine, reciprocal + broadcast normalize on DVE, output DMA on a
  rotating ring) is emitted inline per pass so it overlaps the main
  stream and only the last pass's epilogue sits in the tail.
* Head-0 k/q loads are split into progressively larger slices so the
  first matmul starts as soon as ~96KB has landed instead of 2MB.
* Sparsity handling (host-compiled schedule: chunk skipping, q-range
  trimming, boundary masks, fully-masked-row patching) as in v1.
"""

import numpy as np

import concourse.mybir as mybir
import concourse.tile as tile
from concourse import bacc
from concourse.bass_utils import run_bass_kernel_spmd

F32 = mybir.dt.float32
F32R = mybir.dt.float32r
F16 = mybir.dt.float16
I32 = mybir.dt.int32
U32 = mybir.dt.uint32

B, H, N, D = 2, 16, 2048, 64
NCORES = 8
HPC = (B * H) // NCORES        # heads per core
CHUNK = 128                    # k-chunk (partition dim of S^T)
QP = 1024                      # q extent per pass
NPASS = N // QP
NCHUNK = N // CHUNK
MMF = 512                      # max matmul moving free dim

# exp split: cost-per-column weights for the greedy assignment
ACT_COST = 0.84                # ACT exact exp, ns/col
DVE_COST = 2.08                # DVE affine + correction path, ns/col
DVE_PRELOAD = 8400.0           # DVE's other duties, in equivalent columns

# Schraudolph + quadratic correction constants
LOG2E = float(np.log2(np.e))
SCHRAU_A = float(2.0 ** 23)
SCHRAU_B = float(127 * 2 ** 23)
EXP_CORR_C = (0.01970297198527479, 0.28223653719876435, 1.8137994519512113)

EXP_CORR_NAME = "EXP_SCHRAU_CORR"


def _exp_corr_reference(in0, in1, s0, s1, imm2):
    w = in0.astype(np.float32)
    nw = (~w.view(np.int32)).view(np.float32)
    h = w * nw
    return ((h * np.float32(s0) + np.float32(s1)) * h + np.float32(imm2)) * w


def _register_exp_corr():
    """Register the corrected-Schraudolph exp as a custom DVE op."""
    from concourse import dve_ops
    from concourse.dve_spec import (Spec, Src0, C0, C1, C2, Bin, AluOp,
                                    lower, _has_src1)
    from concourse.dve_uop import DveOpSpec

    if EXP_CORR_NAME in dve_ops._SUB_OPCODE_FOR_NAME:
        for op in dve_ops.OPS:
            if op.name == EXP_CORR_NAME:
                return op
    _not = Bin(AluOp.BITWISE_NOT, Src0, Src0)
    h = Src0 * _not
    body = ((h * C0 + C1) * h + C2) * Src0
    spec = Spec(body=body, reference=_exp_corr_reference)
    shas = {}
    for ver in ("v3", "v4"):
        d = DveOpSpec(name=EXP_CORR_NAME, opcode=0, uops=lower(spec, ver=ver),
                      rd1_en=_has_src1(spec))
        shas[ver] = d.sha(ver)
    op = dve_ops.DveOp(EXP_CORR_NAME, spec, subdim=False, uops_sha=shas)
    dve_ops.OPS.append(op)
    dve_ops._SUB_OPCODE_FOR_NAME[EXP_CORR_NAME] = (
        dve_ops._CUSTOM_DVE_ROW_BASE + len(dve_ops.OPS) - 1)
    dve_ops.CUSTOM_DVE_SPECS[EXP_CORR_NAME] = spec
    return op


def _runs(mask):
    """Maximal [a, b) runs of True in a 1-D bool array."""
    idx = np.flatnonzero(np.diff(np.concatenate(([False], mask, [False])).astype(np.int8)))
    return list(zip(idx[0::2], idx[1::2]))


def _schedule(starts, ends):
    """Per (pass, chunk) work description, shared by all heads/cores."""
    sched = []
    for p in range(NPASS):
        qb = p * QP
        ps = starts[qb:qb + QP]
        pe = ends[qb:qb + QP]
        chunks = []
        for c in range(NCHUNK):
            lo, hi = c * CHUNK, (c + 1) * CHUNK
            allowed = (pe > lo) & (ps < hi)
            if not allowed.any():
                continue
            dis = _runs(~allowed)
            # trim leading/trailing fully-disallowed cols out of S/exp.
            # matmuls want even free offsets/counts, so snap outward and
            # zero the extra disallowed column(s) explicitly.
            qa = dis[0][1] if dis and dis[0][0] == 0 else 0
            qz = dis[-1][0] if dis and dis[-1][1] == QP else QP
            qa_e, qz_e = int(qa) & ~1, min(QP, (int(qz) + 1) & ~1)
            me = _runs(allowed & (pe > lo) & (pe < hi))
            ms = _runs(allowed & (ps > lo) & (ps < hi))
            # interior disallowed spans (inside [qa, qz)) are read by the
            # trimmed PV matmul and must be zeroed; the leading/trailing
            # spans only matter for the first chunk, whose PV is full-width
            interior = [(int(a), int(b)) for a, b in dis if a != 0 and b != QP]
            for a, b in ((qa_e, qa), (qz, qz_e)):
                if a < b:
                    interior.append((int(a), int(b)))
            qa, qz = qa_e, qz_e
            chunks.append(dict(c=c, qa=int(qa), qz=int(qz),
                               memsets=[(int(a), int(b)) for a, b in dis],
                               interior=interior,
                               mule=[(int(a), int(b)) for a, b in me],
                               muls=[(int(a), int(b)) for a, b in ms]))
        sched.append(chunks)
    return sched


# progressive slices for head 0 (units: chunks for kt, MMF cols for qt)
KT0_SLICES = [(0, 1), (1, 2), (2, 4), (4, 8), (8, 16)]
QT0_SLICES = [(0, 1), (1, 2), (2, 4)]


def _build_program(sched, sm_scale, use_me, use_ms):
    exp_op = _register_exp_corr()
    nc = bacc.Bacc("TRN2", target_bir_lowering=False, debug=True)

    # head 0's kt/qt are packed part-contiguous in flat buffers so the
    # startup slices DMA with full-width descriptors; heads 1-3 load whole
    kt0_h = nc.declare_dram_parameter("kt0", [128 * N], F32R, isOutput=False)
    qt0_h = nc.declare_dram_parameter("qt0", [128 * N], F32R, isOutput=False)
    kt_h = nc.declare_dram_parameter("kt", [HPC, 128, N], F32R, isOutput=False)
    qt_h = nc.declare_dram_parameter("qt", [HPC, 128, N], F32R, isOutput=False)
    ve_h = nc.declare_dram_parameter("ve", [HPC, 128, NCHUNK * (D + 1)], F32R, isOutput=False)
    me_h = nc.declare_dram_parameter("me", [128, N], F16, isOutput=False)
    ms_h = nc.declare_dram_parameter("ms", [128, N], F16, isOutput=False)
    id_h = nc.declare_dram_parameter("ident", [D + 1, D + 2], F32R, isOutput=False)
    o_h = nc.declare_dram_parameter("o", [HPC, N, D], F32, isOutput=True)

    exp_f = mybir.ActivationFunctionType.Exp
    mul_op = mybir.AluOpType.mult
    add_op = mybir.AluOpType.add

    a_pool = SCHRAU_A * LOG2E * sm_scale
    c0, c1, c2 = EXP_CORR_C

    with tile.TileContext(nc) as tc:
        with (
            tc.tile_pool(name="singles", bufs=1) as singles,
            tc.tile_pool(name="heads", bufs=1) as heads,
            tc.tile_pool(name="pbuf", bufs=1) as pbuf,
            tc.tile_pool(name="fin", bufs=1) as fin,
            tc.tile_pool(name="psum", bufs=1, space="PSUM") as psum,
        ):
            items = []
            for g in range(HPC):
                for p in range(NPASS):
                    chunks = sched[p]
                    for idx, ch in enumerate(chunks):
                        items.append(dict(g=g, p=p, ch=ch, first=idx == 0,
                                          last=idx == len(chunks) - 1))

            # greedy engine split for exp by weighted column cost
            cost = [0.0, DVE_PRELOAD]     # ACT, DVE-path
            for it in items:
                cols = it["ch"]["qz"] - it["ch"]["qa"]
                if cost[0] * ACT_COST <= cost[1] * DVE_COST:
                    it["eng"] = "act"
                    cost[0] += cols
                else:
                    it["eng"] = "dve"
                    cost[1] += cols

            head_sb = {}

            def load_head(g):
                if g == 0:
                    kt_parts, qt_parts = [], []
                    koff = 0
                    for i, (a, b) in enumerate(KT0_SLICES):
                        w = (b - a) * CHUNK
                        t = heads.tile([128, w], F32R,
                                       tag=f"kt0_{i}", name=f"kt0_{i}")
                        nc.sync.dma_start(
                            out=t,
                            in_=kt0_h[koff:koff + 128 * w].rearrange(
                                "(p x) -> p x", p=128))
                        koff += 128 * w
                        kt_parts.append((a * CHUNK, b * CHUNK, t))
                        if i == 0:
                            q = heads.tile([128, MMF], F32R, tag="qt0_0",
                                           name="qt0_0")
                            nc.scalar.dma_start(
                                out=q,
                                in_=qt0_h[0:128 * MMF].rearrange(
                                    "(p x) -> p x", p=128))
                            qt_parts.append((0, MMF, q))
                    qoff = 128 * MMF
                    for i, (a, b) in enumerate(QT0_SLICES[1:], 1):
                        w = (b - a) * MMF
                        q = heads.tile([128, w], F32R,
                                       tag=f"qt0_{i}", name=f"qt0_{i}")
                        nc.scalar.dma_start(
                            out=q,
                            in_=qt0_h[qoff:qoff + 128 * w].rearrange(
                                "(p x) -> p x", p=128))
                        qoff += 128 * w
                        qt_parts.append((a * MMF, b * MMF, q))
                else:
                    kt_sb = heads.tile([128, N], F32R, tag="kt", bufs=3,
                                       name=f"kt_{g}")
                    nc.sync.dma_start(out=kt_sb, in_=kt_h[g, :, :])
                    qt_sb = heads.tile([128, N], F32R, tag="qt", bufs=3,
                                       name=f"qt_{g}")
                    nc.sync.dma_start(out=qt_sb, in_=qt_h[g, :, :])
                    kt_parts = [(0, N, kt_sb)]
                    qt_parts = [(0, N, qt_sb)]
                ve_sb = heads.tile([128, NCHUNK * (D + 1)], F32R, tag="ve",
                                   bufs=3, name=f"ve_{g}")
                nc.gpsimd.dma_start(out=ve_sb, in_=ve_h[g, :, :])
                head_sb[g] = (kt_parts, qt_parts, ve_sb)

            def kt_ap(g, pp, c):
                for a, b, t in head_sb[g][0]:
                    if a <= c * CHUNK < b:
                        off = c * CHUNK - a
                        return t[pp:pp + 64, off:off + CHUNK]
                raise AssertionError

            def qt_ap(g, pp, p, lo, hi):
                qb = p * QP
                for a, b, t in head_sb[g][1]:
                    if a <= qb + lo < b:
                        return t[pp:pp + 64, qb + lo - a:qb + hi - a]
                raise AssertionError

            o_tiles = {}
            n_out = [0]
            out_rings = [nc.sync, nc.gpsimd, nc.sync, nc.scalar]

            def end_phase(g, p, f_sb, id_sb):
                qb = p * QP
                NT = QP // 128
                # per-transpose pitch padded to 128 f32 (512B) so no single
                # transpose output crosses a 2KB PSUM bank line (a crossing
                # write corrupts the tail of the transpose)
                t_ps = psum.tile([128, NT * 128], F32R, tag="o", bufs=1,
                                 name=f"t_{g}_{p}")
                for t in range(NT):
                    # D+2 output cols: f32r transpose needs an even
                    # innermost count; the extra identity column is zero
                    nc.tensor.transpose(
                        t_ps[:, t * 128:t * 128 + (D + 2)],
                        f_sb[:, t * 128:(t + 1) * 128],
                        id_sb)
                r_sb = fin.tile([128, NT], F32, tag="r", bufs=4,
                                name=f"r_{g}_{p}")
                t3 = t_ps.bitcast(F32).rearrange("q (t c) -> q t c", c=128)
                nc.vector.reciprocal(r_sb, t3[:, :, D])
                oo_sb = fin.tile([128, NT * D], F32, tag="oo", bufs=4,
                                 name=f"oo_{g}_{p}")
                nc.vector.tensor_tensor(
                    oo_sb.rearrange("q (t d) -> q t d", d=D),
                    t3[:, :, :D], r_sb.to_broadcast([128, NT, D]), mul_op)
                eng = out_rings[n_out[0] % 4]
                n_out[0] += 1
                eng.dma_start(
                    out=o_h[g, qb:qb + QP, :].rearrange("(t p) d -> p t d", p=128),
                    in_=oo_sb.rearrange("p (t d) -> p t d", d=D),
                )

            def emit_pv(it, p_sb, id_sb):
                g, p, ch = it["g"], it["p"], it["ch"]
                if (g, p) not in o_tiles:
                    o_tiles[(g, p)] = psum.tile([D + 1, QP], F32, tag="o",
                                                bufs=1, name=f"o_{g}_{p}")
                o_ps = o_tiles[(g, p)]
                ve_sb = head_sb[g][2]
                c = ch["c"]
                for a in range(0, QP, MMF):
                    if it["first"]:
                        lo, hi = a, a + MMF
                    else:
                        lo, hi = max(a, ch["qa"]), min(a + MMF, ch["qz"])
                    if lo < hi:
                        nc.tensor.matmul(
                            o_ps[:, lo:hi],
                            lhsT=ve_sb[:, c * (D + 1):(c + 1) * (D + 1)],
                            rhs=p_sb[:, lo:hi],
                            start=it["first"], stop=it["last"],
                        )
                if it["last"]:
                    f_sb = fin.tile([D + 1, QP], F32R, tag="f", bufs=3,
                                    name=f"f_{g}_{p}")
                    nc.vector.tensor_copy(f_sb, o_ps.bitcast(F32R))
                    del o_tiles[(g, p)]
                    end_phase(g, p, f_sb, id_sb)

            # head 0's tensors gate the first matmuls - their DMAs go first
            load_head(0)
            id_sb = singles.tile([D + 1, D + 2], F32R, tag="ident")
            nc.sync.dma_start(out=id_sb, in_=id_h[:, :])
            me_sb = ms_sb = None
            if use_me:
                me_sb = singles.tile([128, N], F16, tag="me")
                nc.gpsimd.dma_start(out=me_sb, in_=me_h[:, :])
            if use_ms:
                ms_sb = singles.tile([128, N], F16, tag="ms")
                nc.gpsimd.dma_start(out=ms_sb, in_=ms_h[:, :])

            pending = []
            for j0 in range(0, len(items), 2):
                pair = items[j0:j0 + 2]
                # stagger head loads: kick off head g+1's DMAs as soon as
                # head g's first pair is in flight
                g_hi = max(it["g"] for it in pair)
                if g_hi + 1 < HPC and g_hi + 1 not in head_sb:
                    load_head(g_hi + 1)
                tiles = []
                # QK matmuls, interleaved across the pair for row-group
                # concurrency (pp = 0 / 64)
                sub = []
                for k, it in enumerate(pair):
                    ch = it["ch"]
                    s_ps = psum.tile([128, QP], F32, tag="s", bufs=3,
                                     name=f"s_{j0}_{k}")
                    tiles.append(s_ps)
                    mms = []
                    for a in range(0, QP, MMF):
                        lo, hi = max(a, ch["qa"]), min(a + MMF, ch["qz"])
                        if lo < hi:
                            mms.append((s_ps, 64 * k, it, lo, hi))
                    sub.append(mms)
                for pr in [x for tup in __import__("itertools")
                           .zip_longest(*sub) for x in tup if x]:
                    s_ps, pp, it, lo, hi = pr
                    g, p = it["g"], it["p"]
                    nc.tensor.matmul(
                        s_ps[:, lo:hi],
                        lhsT=kt_ap(g, pp, it["ch"]["c"]),
                        rhs=qt_ap(g, pp, p, lo, hi),
                        start=True, stop=True,
                        tile_position=(pp, 0),
                    )
                cur = []
                for k, it in enumerate(pair):
                    ch = it["ch"]
                    g, p = it["g"], it["p"]
                    qb = p * QP
                    lo, hi = ch["qa"], ch["qz"]
                    p_sb = pbuf.tile([128, QP], F32R, tag="p", bufs=10,
                                     name=f"p_{j0}_{k}")
                    if it["eng"] == "act":
                        nc.scalar.activation(p_sb[:, lo:hi], tiles[k][:, lo:hi],
                                             exp_f, scale=sm_scale)
                    else:
                        w_sb = pbuf.tile([128, QP], F32R, tag="w", bufs=6,
                                         name=f"w_{j0}_{k}")
                        nc.vector.tensor_scalar(
                            w_sb[:, lo:hi].bitcast(I32), tiles[k][:, lo:hi],
                            a_pool, SCHRAU_B, mul_op, add_op)
                        nc.vector._custom_dve(
                            exp_op, out=p_sb[:, lo:hi],
                            in0=w_sb[:, lo:hi].bitcast(F32),
                            s0=c0, s1=c1, imm2=c2)
                    # p_sb is SBUF, so masks/memsets can run on Pool, which
                    # cannot touch PSUM and is otherwise idle
                    for a, b in (ch["memsets"] if it["first"] else ch["interior"]):
                        nc.gpsimd.memset(p_sb[:, a:b].bitcast(U32), 0)
                    for mi, (a, b, m_sb) in enumerate(
                            [(a, b, me_sb) for a, b in ch["mule"]]
                            + [(a, b, ms_sb) for a, b in ch["muls"]]):
                        nc.gpsimd.tensor_mul(p_sb[:, a:b], p_sb[:, a:b],
                                             m_sb[:, qb + a:qb + b])
                    cur.append((it, p_sb))
                for it, p_sb in pending:
                    emit_pv(it, p_sb, id_sb)
                pending = cur
            for it, p_sb in pending:
                emit_pv(it, p_sb, id_sb)

    nc.compile()
    return nc


_CACHE = {}


def _get_program(starts, ends, sm_scale, use_me, use_ms):
    key = (starts.tobytes(), ends.tobytes(), float(sm_scale), use_me, use_ms)
    if key not in _CACHE:
        sched = _schedule(starts, ends)
        _CACHE[key] = _build_program(sched, float(sm_scale), use_me, use_ms)
    return _CACHE[key]


def _prep_inputs(q, k, v, starts, ends, use_me, use_ms):
    """Per-core input dicts."""
    qf = np.asarray(q, np.float32).reshape(B * H, N, D)
    kf = np.asarray(k, np.float32).reshape(B * H, N, D)
    vf = np.asarray(v, np.float32).reshape(B * H, N, D)

    # boundary mask strips (shared across heads): column j holds the
    # within-chunk prefix/suffix mask for row_ends[j]/row_starts[j]
    rows = np.arange(128, dtype=np.int64)[:, None]
    me = (rows < (ends[None, :] % CHUNK)).astype(np.float16)
    ms = (rows >= (starts[None, :] % CHUNK)).astype(np.float16)
    ident = np.zeros((D + 1, D + 2), np.float32)
    ident[:, :D + 1] = np.eye(D + 1, dtype=np.float32)

    in_maps = []
    for i in range(NCORES):
        sl = slice(i * HPC, (i + 1) * HPC)
        kt1 = kf[sl].transpose(0, 2, 1)                      # [HPC, D, N]
        qt1 = qf[sl].transpose(0, 2, 1)
        kt = np.ascontiguousarray(np.concatenate([kt1, kt1], axis=1))
        qt = np.ascontiguousarray(np.concatenate([qt1, qt1], axis=1))
        kt0 = np.concatenate(
            [kt[0][:, a * CHUNK:b * CHUNK].ravel() for a, b in KT0_SLICES])
        qt0 = np.concatenate(
            [qt[0][:, a * MMF:b * MMF].ravel() for a, b in QT0_SLICES])
        ve = np.ones([HPC, 128, NCHUNK, D + 1], np.float32)
        ve[:, :, :, :D] = vf[sl].reshape(HPC, NCHUNK, CHUNK, D).transpose(0, 2, 1, 3)
        ve = np.ascontiguousarray(ve.reshape(HPC, 128, NCHUNK * (D + 1)))
        in_maps.append({"kt": kt, "qt": qt, "kt0": kt0, "qt0": qt0,
                        "ve": ve, "me": me, "ms": ms, "ident": ident})
    return in_maps


def _run(inputs, trace=False):
    q, k, v = inputs["q"], inputs["k"], inputs["v"]
    sm_scale = float(np.asarray(inputs["sm_scale"]))
    starts_raw = np.asarray(inputs["row_starts"], np.int64)
    ends_raw = np.asarray(inputs["row_ends"], np.int64)
    starts = np.clip(starts_raw, 0, N)
    ends = np.clip(ends_raw, 0, N)

    use_ms = bool((starts % CHUNK).any())
    use_me = bool(((ends % CHUNK) * (ends > starts)).any())

    nc = _get_program(starts, ends, sm_scale, use_me, use_ms)
    in_maps = _prep_inputs(q, k, v, starts, ends, use_me, use_ms)
    res = run_bass_kernel_spmd(nc, in_maps, list(range(NCORES)), trace=trace)

    out = np.empty([B * H, N, D], np.float32)
    for i in range(NCORES):
        out[i * HPC:(i + 1) * HPC] = res.results[i]["o"]
    out = out.reshape(B, H, N, D)

    empty = ends <= starts
    if empty.any():
        mean_v = np.asarray(v, np.float32).mean(axis=2)          # [B, H, D]
        out[:, :, empty, :] = mean_v[:, :, None, :]
    return out, res.exec_time_ns


def kernel(**inputs) -> np.ndarray:
    out, _ = _run(inputs, trace=False)
    return out


# revision 19
# speedup vs baseline: 1.1311x; 1.1311x over previous
"""Block-sparse attention kernel for Trainium2 (8 NeuronCores, SPMD).

Strategy (v2)
-------------
* Shard batch*heads (2*16 = 32 pairs) across 8 cores, 4 heads per core.
* Flash-style attention in S^T layout: S^T[k, q] via matmul(lhsT=K^T
  chunk, rhs=Q^T), with q/k inputs pre-transposed on the host in fp16
  and replicated into both partition halves so chunk pairs run as
  concurrent row-tiled K=64 matmuls (PE row groups 0-1 / 2-3).
* exp(sm_scale * S^T) is split across engines: the ACT (scalar) engine
  computes exact exp for a share of the chunks; the rest go through a
  two-instruction approximate path on the DVE - tensor_scalar computes
  the Schraudolph bit-trick i32(A*s + B) (bits of 2^i*(1+f)), and a
  custom DVE op applies a quadratic correction using h = w*bitcast(~w),
  a pure function of the mantissa fraction (max rel err ~0.52%).
  (Pool/GPSIMD cannot read PSUM, so it only gets masks/memsets.)
* The PV matmul contracts the full 128-row chunk (K=128 uses every PE
  row - splitting it into row tiles would just stream each output
  column twice), accumulating O^T + softmax denominators per pass.
* Q-pass granularity is 512 (PSUM: 4 score banks + 2 O banks +
  2 transpose banks = 8). End-phase (transpose O^T -> O on the tensor
  engine, reciprocal + broadcast normalize on DVE, output DMA on a
  rotating ring) is emitted inline per pass so it overlaps the main
  stream and only the last pass's epilogue sits in the tail.
* Head-0 k/q loads are split into progressively larger slices so the
  first matmul starts as soon as ~96KB has landed instead of 2MB.
* Sparsity handling (host-compiled schedule: chunk skipping, q-range
  trimming, boundary masks, fully-masked-row patching) as in v1.
"""

import ml_dtypes
import numpy as np

import concourse.mybir as mybir
import concourse.tile as tile
from concourse import bacc
from concourse.bass_utils import run_bass_kernel_spmd

F32 = mybir.dt.float32
F32R = mybir.dt.float32r
F16 = mybir.dt.float16
BF16 = mybir.dt.bfloat16
I32 = mybir.dt.int32
U32 = mybir.dt.uint32

B, H, N, D = 2, 16, 2048, 64
NCORES = 8
HPC = (B * H) // NCORES        # heads per core
CHUNK = 128                    # k-chunk (partition dim of S^T)
QP = 1024                      # q extent per pass
NPASS = N // QP
NCHUNK = N // CHUNK
MMF = 512                      # max matmul moving free dim

# exp split: cost-per-column weights for the greedy assignment
ACT_COST = 0.84                # ACT exact exp, ns/col
DVE_COST = 2.08                # DVE affine + correction path, ns/col
DVE_PRELOAD = 8400.0           # DVE's other duties, in equivalent columns

# Schraudolph + quadratic correction constants
LOG2E = float(np.log2(np.e))
SCHRAU_A = float(2.0 ** 23)
SCHRAU_B = float(127 * 2 ** 23)
EXP_CORR_C = (0.01970297198527479, 0.28223653719876435, 1.8137994519512113)

EXP_CORR_NAME = "EXP_SCHRAU_CORR"


def _exp_corr_reference(in0, in1, s0, s1, imm2):
    w = in0.astype(np.float32)
    nw = (~w.view(np.int32)).view(np.float32)
    h = w * nw
    return ((h * np.float32(s0) + np.float32(s1)) * h + np.float32(imm2)) * w


def _register_exp_corr():
    """Register the corrected-Schraudolph exp as a custom DVE op."""
    from concourse import dve_ops
    from concourse.dve_spec import (Spec, Src0, C0, C1, C2, Bin, AluOp,
                                    lower, _has_src1)
    from concourse.dve_uop import DveOpSpec

    if EXP_CORR_NAME in dve_ops._SUB_OPCODE_FOR_NAME:
        for op in dve_ops.OPS:
            if op.name == EXP_CORR_NAME:
                return op
    _not = Bin(AluOp.BITWISE_NOT, Src0, Src0)
    h = Src0 * _not
    body = ((h * C0 + C1) * h + C2) * Src0
    spec = Spec(body=body, reference=_exp_corr_reference)
    shas = {}
    for ver in ("v3", "v4"):
        d = DveOpSpec(name=EXP_CORR_NAME, opcode=0, uops=lower(spec, ver=ver),
                      rd1_en=_has_src1(spec))
        shas[ver] = d.sha(ver)
    op = dve_ops.DveOp(EXP_CORR_NAME, spec, subdim=False, uops_sha=shas)
    dve_ops.OPS.append(op)
    dve_ops._SUB_OPCODE_FOR_NAME[EXP_CORR_NAME] = (
        dve_ops._CUSTOM_DVE_ROW_BASE + len(dve_ops.OPS) - 1)
    dve_ops.CUSTOM_DVE_SPECS[EXP_CORR_NAME] = spec
    return op


def _runs(mask):
    """Maximal [a, b) runs of True in a 1-D bool array."""
    idx = np.flatnonzero(np.diff(np.concatenate(([False], mask, [False])).astype(np.int8)))
    return list(zip(idx[0::2], idx[1::2]))


def _schedule(starts, ends):
    """Per (pass, chunk) work description, shared by all heads/cores."""
    sched = []
    for p in range(NPASS):
        qb = p * QP
        ps = starts[qb:qb + QP]
        pe = ends[qb:qb + QP]
        chunks = []
        for c in range(NCHUNK):
            lo, hi = c * CHUNK, (c + 1) * CHUNK
            allowed = (pe > lo) & (ps < hi)
            if not allowed.any():
                continue
            dis = _runs(~allowed)
            # trim leading/trailing fully-disallowed cols out of S/exp.
            # matmuls want even free offsets/counts, so snap outward and
            # zero the extra disallowed column(s) explicitly.
            qa = dis[0][1] if dis and dis[0][0] == 0 else 0
            qz = dis[-1][0] if dis and dis[-1][1] == QP else QP
            qa_e, qz_e = int(qa) & ~1, min(QP, (int(qz) + 1) & ~1)
            me = _runs(allowed & (pe > lo) & (pe < hi))
            ms = _runs(allowed & (ps > lo) & (ps < hi))
            # interior disallowed spans (inside [qa, qz)) are read by the
            # trimmed PV matmul and must be zeroed; the leading/trailing
            # spans only matter for the first chunk, whose PV is full-width
            interior = [(int(a), int(b)) for a, b in dis if a != 0 and b != QP]
            for a, b in ((qa_e, qa), (qz, qz_e)):
                if a < b:
                    interior.append((int(a), int(b)))
            qa, qz = qa_e, qz_e
            chunks.append(dict(c=c, qa=int(qa), qz=int(qz),
                               memsets=[(int(a), int(b)) for a, b in dis],
                               interior=interior,
                               mule=[(int(a), int(b)) for a, b in me],
                               muls=[(int(a), int(b)) for a, b in ms]))
        sched.append(chunks)
    return sched


# progressive slices for head 0 (units: chunks for kt, MMF cols for qt)
KT0_SLICES = [(0, 1), (1, 2), (2, 4), (4, 8), (8, 16)]
QT0_SLICES = [(0, 1), (1, 2), (2, 4)]


def _build_program(sched, sm_scale, use_me, use_ms):
    exp_op = _register_exp_corr()
    nc = bacc.Bacc("TRN2", target_bir_lowering=False, debug=True)

    # head 0's kt/qt are packed part-contiguous in flat buffers so the
    # startup slices DMA with full-width descriptors; heads 1-3 load whole
    kt0_h = nc.declare_dram_parameter("kt0", [128 * N], F16, isOutput=False)
    qt0_h = nc.declare_dram_parameter("qt0", [128 * N], F16, isOutput=False)
    kt_h = nc.declare_dram_parameter("kt", [HPC, 128, N], F16, isOutput=False)
    qt_h = nc.declare_dram_parameter("qt", [HPC, 128, N], F16, isOutput=False)
    ve_h = nc.declare_dram_parameter("ve", [HPC, 128, NCHUNK * (D + 1)], BF16, isOutput=False)
    me_h = nc.declare_dram_parameter("me", [128, N], BF16, isOutput=False)
    ms_h = nc.declare_dram_parameter("ms", [128, N], BF16, isOutput=False)
    id_h = nc.declare_dram_parameter("ident", [D + 1, D + 2], F32R, isOutput=False)
    o_h = nc.declare_dram_parameter("o", [HPC, N, D], F32, isOutput=True)

    exp_f = mybir.ActivationFunctionType.Exp
    mul_op = mybir.AluOpType.mult
    add_op = mybir.AluOpType.add

    a_pool = SCHRAU_A * LOG2E * sm_scale
    c0, c1, c2 = EXP_CORR_C

    with tile.TileContext(nc) as tc:
        with (
            tc.tile_pool(name="singles", bufs=1) as singles,
            tc.tile_pool(name="heads", bufs=1) as heads,
            tc.tile_pool(name="pbuf", bufs=1) as pbuf,
            tc.tile_pool(name="fin", bufs=1) as fin,
            tc.tile_pool(name="psum", bufs=1, space="PSUM") as psum,
        ):
            items = []
            for g in range(HPC):
                for p in range(NPASS):
                    chunks = sched[p]
                    for idx, ch in enumerate(chunks):
                        items.append(dict(g=g, p=p, ch=ch, first=idx == 0,
                                          last=idx == len(chunks) - 1))

            # greedy engine split for exp by weighted column cost
            cost = [0.0, DVE_PRELOAD]     # ACT, DVE-path
            for it in items:
                cols = it["ch"]["qz"] - it["ch"]["qa"]
                if cost[0] * ACT_COST <= cost[1] * DVE_COST:
                    it["eng"] = "act"
                    cost[0] += cols
                else:
                    it["eng"] = "dve"
                    cost[1] += cols

            head_sb = {}

            def load_head(g):
                if g == 0:
                    kt_parts, qt_parts = [], []
                    koff = 0
                    for i, (a, b) in enumerate(KT0_SLICES):
                        w = (b - a) * CHUNK
                        t = heads.tile([128, w], F16,
                                       tag=f"kt0_{i}", name=f"kt0_{i}")
                        nc.sync.dma_start(
                            out=t,
                            in_=kt0_h[koff:koff + 128 * w].rearrange(
                                "(p x) -> p x", p=128))
                        koff += 128 * w
                        kt_parts.append((a * CHUNK, b * CHUNK, t))
                        if i == 0:
                            q = heads.tile([128, MMF], F16, tag="qt0_0",
                                           name="qt0_0")
                            nc.scalar.dma_start(
                                out=q,
                                in_=qt0_h[0:128 * MMF].rearrange(
                                    "(p x) -> p x", p=128))
                            qt_parts.append((0, MMF, q))
                    qoff = 128 * MMF
                    for i, (a, b) in enumerate(QT0_SLICES[1:], 1):
                        w = (b - a) * MMF
                        q = heads.tile([128, w], F16,
                                       tag=f"qt0_{i}", name=f"qt0_{i}")
                        nc.scalar.dma_start(
                            out=q,
                            in_=qt0_h[qoff:qoff + 128 * w].rearrange(
                                "(p x) -> p x", p=128))
                        qoff += 128 * w
                        qt_parts.append((a * MMF, b * MMF, q))
                else:
                    kt_sb = heads.tile([128, N], F16, tag="kt", bufs=3,
                                       name=f"kt_{g}")
                    nc.sync.dma_start(out=kt_sb, in_=kt_h[g, :, :])
                    qt_sb = heads.tile([128, N], F16, tag="qt", bufs=3,
                                       name=f"qt_{g}")
                    nc.sync.dma_start(out=qt_sb, in_=qt_h[g, :, :])
                    kt_parts = [(0, N, kt_sb)]
                    qt_parts = [(0, N, qt_sb)]
                ve_sb = heads.tile([128, NCHUNK * (D + 1)], BF16, tag="ve",
                                   bufs=3, name=f"ve_{g}")
                nc.gpsimd.dma_start(out=ve_sb, in_=ve_h[g, :, :])
                head_sb[g] = (kt_parts, qt_parts, ve_sb)

            def kt_ap(g, pp, c):
                for a, b, t in head_sb[g][0]:
                    if a <= c * CHUNK < b:
                        off = c * CHUNK - a
                        return t[pp:pp + 64, off:off + CHUNK]
                raise AssertionError

            def qt_ap(g, pp, p, lo, hi):
                qb = p * QP
                for a, b, t in head_sb[g][1]:
                    if a <= qb + lo < b:
                        return t[pp:pp + 64, qb + lo - a:qb + hi - a]
                raise AssertionError

            o_tiles = {}
            n_out = [0]
            out_rings = [nc.sync, nc.gpsimd, nc.sync, nc.scalar]

            def end_phase(g, p, f_sb, id_sb):
                qb = p * QP
                NT = QP // 128
                # per-transpose pitch padded to 128 f32 (512B) so no single
                # transpose output crosses a 2KB PSUM bank line (a crossing
                # write corrupts the tail of the transpose)
                t_ps = psum.tile([128, NT * 128], F32R, tag="o", bufs=1,
                                 name=f"t_{g}_{p}")
                for t in range(NT):
                    # D+2 output cols: f32r transpose needs an even
                    # innermost count; the extra identity column is zero
                    nc.tensor.transpose(
                        t_ps[:, t * 128:t * 128 + (D + 2)],
                        f_sb[:, t * 128:(t + 1) * 128],
                        id_sb)
                r_sb = fin.tile([128, NT], F32, tag="r", bufs=4,
                                name=f"r_{g}_{p}")
                t3 = t_ps.bitcast(F32).rearrange("q (t c) -> q t c", c=128)
                nc.vector.reciprocal(r_sb, t3[:, :, D])
                oo_sb = fin.tile([128, NT * D], F32, tag="oo", bufs=4,
                                 name=f"oo_{g}_{p}")
                nc.vector.tensor_tensor(
                    oo_sb.rearrange("q (t d) -> q t d", d=D),
                    t3[:, :, :D], r_sb.to_broadcast([128, NT, D]), mul_op)
                eng = out_rings[n_out[0] % 4]
                n_out[0] += 1
                eng.dma_start(
                    out=o_h[g, qb:qb + QP, :].rearrange("(t p) d -> p t d", p=128),
                    in_=oo_sb.rearrange("p (t d) -> p t d", d=D),
                )

            def emit_pv(it, p_sb, id_sb):
                g, p, ch = it["g"], it["p"], it["ch"]
                if (g, p) not in o_tiles:
                    o_tiles[(g, p)] = psum.tile([D + 1, QP], F32, tag="o",
                                                bufs=1, name=f"o_{g}_{p}")
                o_ps = o_tiles[(g, p)]
                ve_sb = head_sb[g][2]
                c = ch["c"]
                for a in range(0, QP, MMF):
                    if it["first"]:
                        lo, hi = a, a + MMF
                    else:
                        lo, hi = max(a, ch["qa"]), min(a + MMF, ch["qz"])
                    if lo < hi:
                        nc.tensor.matmul(
                            o_ps[:, lo:hi],
                            lhsT=ve_sb[:, c * (D + 1):(c + 1) * (D + 1)],
                            rhs=p_sb[:, lo:hi],
                            start=it["first"], stop=it["last"],
                        )
                if it["last"]:
                    f_sb = fin.tile([D + 1, QP], F32R, tag="f", bufs=3,
                                    name=f"f_{g}_{p}")
                    nc.vector.tensor_copy(f_sb, o_ps.bitcast(F32R))
                    del o_tiles[(g, p)]
                    end_phase(g, p, f_sb, id_sb)

            # head 0's tensors gate the first matmuls - their DMAs go first
            load_head(0)
            id_sb = singles.tile([D + 1, D + 2], F32R, tag="ident")
            nc.sync.dma_start(out=id_sb, in_=id_h[:, :])
            me_sb = ms_sb = None
            if use_me:
                me_sb = singles.tile([128, N], BF16, tag="me")
                nc.gpsimd.dma_start(out=me_sb, in_=me_h[:, :])
            if use_ms:
                ms_sb = singles.tile([128, N], BF16, tag="ms")
                nc.gpsimd.dma_start(out=ms_sb, in_=ms_h[:, :])

            pending = []
            for j0 in range(0, len(items), 2):
                pair = items[j0:j0 + 2]
                # stagger head loads: kick off head g+1's DMAs as soon as
                # head g's first pair is in flight
                g_hi = max(it["g"] for it in pair)
                if g_hi + 1 < HPC and g_hi + 1 not in head_sb:
                    load_head(g_hi + 1)
                tiles = []
                # QK matmuls, interleaved across the pair for row-group
                # concurrency (pp = 0 / 64)
                sub = []
                for k, it in enumerate(pair):
                    ch = it["ch"]
                    s_ps = psum.tile([128, QP], F32, tag="s", bufs=3,
                                     name=f"s_{j0}_{k}")
                    tiles.append(s_ps)
                    mms = []
                    for a in range(0, QP, MMF):
                        lo, hi = max(a, ch["qa"]), min(a + MMF, ch["qz"])
                        if lo < hi:
                            mms.append((s_ps, 64 * k, it, lo, hi))
                    sub.append(mms)
                for pr in [x for tup in __import__("itertools")
                           .zip_longest(*sub) for x in tup if x]:
                    s_ps, pp, it, lo, hi = pr
                    g, p = it["g"], it["p"]
                    nc.tensor.matmul(
                        s_ps[:, lo:hi],
                        lhsT=kt_ap(g, pp, it["ch"]["c"]),
                        rhs=qt_ap(g, pp, p, lo, hi),
                        start=True, stop=True,
                        tile_position=(pp, 0),
                    )
                cur = []
                for k, it in enumerate(pair):
                    ch = it["ch"]
                    g, p = it["g"], it["p"]
                    qb = p * QP
                    lo, hi = ch["qa"], ch["qz"]
                    p_sb = pbuf.tile([128, QP], BF16, tag="p", bufs=10,
                                     name=f"p_{j0}_{k}")
                    if it["eng"] == "act":
                        nc.scalar.activation(p_sb[:, lo:hi], tiles[k][:, lo:hi],
                                             exp_f, scale=sm_scale)
                    else:
                        w_sb = pbuf.tile([128, QP], F32R, tag="w", bufs=6,
                                         name=f"w_{j0}_{k}")
                        nc.vector.tensor_scalar(
                            w_sb[:, lo:hi].bitcast(I32), tiles[k][:, lo:hi],
                            a_pool, SCHRAU_B, mul_op, add_op)
                        nc.vector._custom_dve(
                            exp_op, out=p_sb[:, lo:hi],
                            in0=w_sb[:, lo:hi].bitcast(F32),
                            s0=c0, s1=c1, imm2=c2)
                    # p_sb is SBUF, so masks/memsets can run on Pool, which
                    # cannot touch PSUM and is otherwise idle
                    for a, b in (ch["memsets"] if it["first"] else ch["interior"]):
                        nc.gpsimd.memset(p_sb[:, a:b].bitcast(mybir.dt.uint16), 0)
                    for mi, (a, b, m_sb) in enumerate(
                            [(a, b, me_sb) for a, b in ch["mule"]]
                            + [(a, b, ms_sb) for a, b in ch["muls"]]):
                        nc.gpsimd.tensor_mul(p_sb[:, a:b], p_sb[:, a:b],
                                             m_sb[:, qb + a:qb + b])
                    cur.append((it, p_sb))
                for it, p_sb in pending:
                    emit_pv(it, p_sb, id_sb)
                pending = cur
            for it, p_sb in pending:
                emit_pv(it, p_sb, id_sb)

    nc.compile()
    return nc


_CACHE = {}


def _get_program(starts, ends, sm_scale, use_me, use_ms):
    key = (starts.tobytes(), ends.tobytes(), float(sm_scale), use_me, use_ms)
    if key not in _CACHE:
        sched = _schedule(starts, ends)
        _CACHE[key] = _build_program(sched, float(sm_scale), use_me, use_ms)
    return _CACHE[key]


def _prep_inputs(q, k, v, starts, ends, use_me, use_ms):
    """Per-core input dicts."""
    qf = np.asarray(q, np.float32).reshape(B * H, N, D)
    kf = np.asarray(k, np.float32).reshape(B * H, N, D)
    vf = np.asarray(v, np.float32).reshape(B * H, N, D)

    # boundary mask strips (shared across heads): column j holds the
    # within-chunk prefix/suffix mask for row_ends[j]/row_starts[j]
    rows = np.arange(128, dtype=np.int64)[:, None]
    me = (rows < (ends[None, :] % CHUNK)).astype(ml_dtypes.bfloat16)
    ms = (rows >= (starts[None, :] % CHUNK)).astype(ml_dtypes.bfloat16)
    ident = np.zeros((D + 1, D + 2), np.float32)
    ident[:, :D + 1] = np.eye(D + 1, dtype=np.float32)

    in_maps = []
    for i in range(NCORES):
        sl = slice(i * HPC, (i + 1) * HPC)
        kt1 = kf[sl].transpose(0, 2, 1).astype(np.float16)   # [HPC, D, N]
        qt1 = qf[sl].transpose(0, 2, 1).astype(np.float16)
        kt = np.ascontiguousarray(np.concatenate([kt1, kt1], axis=1))
        qt = np.ascontiguousarray(np.concatenate([qt1, qt1], axis=1))
        kt0 = np.concatenate(
            [kt[0][:, a * CHUNK:b * CHUNK].ravel() for a, b in KT0_SLICES])
        qt0 = np.concatenate(
            [qt[0][:, a * MMF:b * MMF].ravel() for a, b in QT0_SLICES])
        ve = np.ones([HPC, 128, NCHUNK, D + 1], ml_dtypes.bfloat16)
        ve[:, :, :, :D] = vf[sl].reshape(HPC, NCHUNK, CHUNK, D).transpose(0, 2, 1, 3)
        ve = np.ascontiguousarray(ve.reshape(HPC, 128, NCHUNK * (D + 1)))
        in_maps.append({"kt": kt, "qt": qt, "kt0": kt0, "qt0": qt0,
                        "ve": ve, "me": me, "ms": ms, "ident": ident})
    return in_maps


def _run(inputs, trace=False):
    q, k, v = inputs["q"], inputs["k"], inputs["v"]
    sm_scale = float(np.asarray(inputs["sm_scale"]))
    starts_raw = np.asarray(inputs["row_starts"], np.int64)
    ends_raw = np.asarray(inputs["row_ends"], np.int64)
    starts = np.clip(starts_raw, 0, N)
    ends = np.clip(ends_raw, 0, N)

    use_ms = bool((starts % CHUNK).any())
    use_me = bool(((ends % CHUNK) * (ends > starts)).any())

    nc = _get_program(starts, ends, sm_scale, use_me, use_ms)
    in_maps = _prep_inputs(q, k, v, starts, ends, use_me, use_ms)
    res = run_bass_kernel_spmd(nc, in_maps, list(range(NCORES)), trace=trace)

    out = np.empty([B * H, N, D], np.float32)
    for i in range(NCORES):
        out[i * HPC:(i + 1) * HPC] = res.results[i]["o"]
    out = out.reshape(B, H, N, D)

    empty = ends <= starts
    if empty.any():
        mean_v = np.asarray(v, np.float32).mean(axis=2)          # [B, H, D]
        out[:, :, empty, :] = mean_v[:, :, None, :]
    return out, res.exec_time_ns


def kernel(**inputs) -> np.ndarray:
    out, _ = _run(inputs, trace=False)
    return out


# revision 21
# speedup vs baseline: 1.1923x; 1.0542x over previous
"""Block-sparse attention kernel for Trainium2 (8 NeuronCores, SPMD).

Strategy (v2)
-------------
* Shard batch*heads (2*16 = 32 pairs) across 8 cores, 4 heads per core.
* Flash-style attention in S^T layout: S^T[k, q] via matmul(lhsT=K^T
  chunk, rhs=Q^T), with q/k inputs pre-transposed on the host in fp16
  and replicated into both partition halves so chunk pairs run as
  concurrent row-tiled K=64 matmuls (PE row groups 0-1 / 2-3).
* exp(sm_scale * S^T) is split across engines: the ACT (scalar) engine
  computes exact exp for a share of the chunks; the rest go through a
  two-instruction approximate path on the DVE - tensor_scalar computes
  the Schraudolph bit-trick i32(A*s + B) (bits of 2^i*(1+f)), and a
  custom DVE op applies a quadratic correction using h = w*bitcast(~w),
  a pure function of the mantissa fraction (max rel err ~0.52%).
  (Pool/GPSIMD cannot read PSUM, so it only gets masks/memsets.)
* The PV matmul contracts the full 128-row chunk (K=128 uses every PE
  row - splitting it into row tiles would just stream each output
  column twice), accumulating O^T + softmax denominators per pass.
* Q-pass granularity is 512 (PSUM: 4 score banks + 2 O banks +
  2 transpose banks = 8). End-phase (transpose O^T -> O on the tensor
  engine, reciprocal + broadcast normalize on DVE, output DMA on a
  rotating ring) is emitted inline per pass so it overlaps the main
  stream and only the last pass's epilogue sits in the tail.
* Head-0 k/q loads are split into progressively larger slices so the
  first matmul starts as soon as ~96KB has landed instead of 2MB.
* Sparsity handling (host-compiled schedule: chunk skipping, q-range
  trimming, boundary masks, fully-masked-row patching) as in v1.
"""

import ml_dtypes
import numpy as np

import concourse.mybir as mybir
import concourse.tile as tile
from concourse import bacc
from concourse.bass_utils import run_bass_kernel_spmd

F32 = mybir.dt.float32
F32R = mybir.dt.float32r
F16 = mybir.dt.float16
BF16 = mybir.dt.bfloat16
I32 = mybir.dt.int32
U32 = mybir.dt.uint32

B, H, N, D = 2, 16, 2048, 64
NCORES = 8
HPC = (B * H) // NCORES        # heads per core
CHUNK = 128                    # k-chunk (partition dim of S^T)
QP = 1024                      # q extent per pass
NPASS = N // QP
NCHUNK = N // CHUNK
MMF = 512                      # max matmul moving free dim

# exp split: cost-per-column weights for the greedy assignment
ACT_COST = 0.84                # ACT exact exp, ns/col
DVE_COST = 2.08                # DVE affine + correction path, ns/col
DVE_PRELOAD = 8400.0           # DVE's other duties, in equivalent columns

# Schraudolph + quadratic correction constants
LOG2E = float(np.log2(np.e))
SCHRAU_A = float(2.0 ** 23)
SCHRAU_B = float(127 * 2 ** 23)
EXP_CORR_C = (0.01970297198527479, 0.28223653719876435, 1.8137994519512113)

EXP_CORR_NAME = "EXP_SCHRAU_CORR"


def _exp_corr_reference(in0, in1, s0, s1, imm2):
    w = in0.astype(np.float32)
    nw = (~w.view(np.int32)).view(np.float32)
    h = w * nw
    return ((h * np.float32(s0) + np.float32(s1)) * h + np.float32(imm2)) * w


def _register_exp_corr():
    """Register the corrected-Schraudolph exp as a custom DVE op."""
    from concourse import dve_ops
    from concourse.dve_spec import (Spec, Src0, C0, C1, C2, Bin, AluOp,
                                    lower, _has_src1)
    from concourse.dve_uop import DveOpSpec

    if EXP_CORR_NAME in dve_ops._SUB_OPCODE_FOR_NAME:
        for op in dve_ops.OPS:
            if op.name == EXP_CORR_NAME:
                return op
    _not = Bin(AluOp.BITWISE_NOT, Src0, Src0)
    h = Src0 * _not
    body = ((h * C0 + C1) * h + C2) * Src0
    spec = Spec(body=body, reference=_exp_corr_reference)
    shas = {}
    for ver in ("v3", "v4"):
        d = DveOpSpec(name=EXP_CORR_NAME, opcode=0, uops=lower(spec, ver=ver),
                      rd1_en=_has_src1(spec))
        shas[ver] = d.sha(ver)
    op = dve_ops.DveOp(EXP_CORR_NAME, spec, subdim=False, uops_sha=shas)
    dve_ops.OPS.append(op)
    dve_ops._SUB_OPCODE_FOR_NAME[EXP_CORR_NAME] = (
        dve_ops._CUSTOM_DVE_ROW_BASE + len(dve_ops.OPS) - 1)
    dve_ops.CUSTOM_DVE_SPECS[EXP_CORR_NAME] = spec
    return op


def _runs(mask):
    """Maximal [a, b) runs of True in a 1-D bool array."""
    idx = np.flatnonzero(np.diff(np.concatenate(([False], mask, [False])).astype(np.int8)))
    return list(zip(idx[0::2], idx[1::2]))


def _schedule(starts, ends):
    """Per (pass, chunk) work description, shared by all heads/cores."""
    sched = []
    for p in range(NPASS):
        qb = p * QP
        ps = starts[qb:qb + QP]
        pe = ends[qb:qb + QP]
        chunks = []
        for c in range(NCHUNK):
            lo, hi = c * CHUNK, (c + 1) * CHUNK
            allowed = (pe > lo) & (ps < hi)
            if not allowed.any():
                continue
            dis = _runs(~allowed)
            # trim leading/trailing fully-disallowed cols out of S/exp.
            # matmuls want even free offsets/counts, so snap outward and
            # zero the extra disallowed column(s) explicitly.
            qa = dis[0][1] if dis and dis[0][0] == 0 else 0
            qz = dis[-1][0] if dis and dis[-1][1] == QP else QP
            qa_e, qz_e = int(qa) & ~1, min(QP, (int(qz) + 1) & ~1)
            me = _runs(allowed & (pe > lo) & (pe < hi))
            ms = _runs(allowed & (ps > lo) & (ps < hi))
            # interior disallowed spans (inside [qa, qz)) are read by the
            # trimmed PV matmul and must be zeroed; the leading/trailing
            # spans only matter for the first chunk, whose PV is full-width
            interior = [(int(a), int(b)) for a, b in dis if a != 0 and b != QP]
            for a, b in ((qa_e, qa), (qz, qz_e)):
                if a < b:
                    interior.append((int(a), int(b)))
            qa, qz = qa_e, qz_e
            chunks.append(dict(c=c, qa=int(qa), qz=int(qz),
                               memsets=[(int(a), int(b)) for a, b in dis],
                               interior=interior,
                               mule=[(int(a), int(b)) for a, b in me],
                               muls=[(int(a), int(b)) for a, b in ms]))
        sched.append(chunks)
    return sched


# progressive slices for head 0 (units: chunks for kt, MMF cols for qt)
KT0_SLICES = [(0, 1), (1, 2), (2, 4), (4, 8), (8, 16)]
QT0_SLICES = [(0, 1), (1, 2), (2, 4)]


def _build_program(sched, sm_scale, use_me, use_ms):
    exp_op = _register_exp_corr()
    nc = bacc.Bacc("TRN2", target_bir_lowering=False, debug=True)

    # head 0's kt/qt are packed part-contiguous in flat buffers so the
    # startup slices DMA with full-width descriptors; heads 1-3 load whole
    kt0_h = nc.declare_dram_parameter("kt0", [128 * N], F16, isOutput=False)
    qt0_h = nc.declare_dram_parameter("qt0", [128 * N], F16, isOutput=False)
    kt_h = nc.declare_dram_parameter("kt", [HPC, 128, N], F16, isOutput=False)
    qt_h = nc.declare_dram_parameter("qt", [HPC, 128, N], F16, isOutput=False)
    ve_h = nc.declare_dram_parameter("ve", [HPC, 128, NCHUNK * (D + 1)], BF16, isOutput=False)
    me_h = nc.declare_dram_parameter("me", [128, N], BF16, isOutput=False)
    ms_h = nc.declare_dram_parameter("ms", [128, N], BF16, isOutput=False)
    id_h = nc.declare_dram_parameter("ident", [D + 1, D + 2], F32R, isOutput=False)
    o_h = nc.declare_dram_parameter("o", [HPC, N, D], F32, isOutput=True)

    exp_f = mybir.ActivationFunctionType.Exp
    mul_op = mybir.AluOpType.mult
    add_op = mybir.AluOpType.add

    a_pool = SCHRAU_A * LOG2E * sm_scale
    c0, c1, c2 = EXP_CORR_C

    with tile.TileContext(nc) as tc:
        with (
            tc.tile_pool(name="singles", bufs=1) as singles,
            tc.tile_pool(name="heads", bufs=1) as heads,
            tc.tile_pool(name="pbuf", bufs=1) as pbuf,
            tc.tile_pool(name="fin", bufs=1) as fin,
            tc.tile_pool(name="psum", bufs=1, space="PSUM") as psum,
        ):
            items = []
            for g in range(HPC):
                for p in range(NPASS):
                    chunks = sched[p]
                    for idx, ch in enumerate(chunks):
                        items.append(dict(g=g, p=p, ch=ch, first=idx == 0,
                                          last=idx == len(chunks) - 1))

            # greedy engine split for exp by weighted column cost
            cost = [0.0, DVE_PRELOAD]     # ACT, DVE-path
            for it in items:
                cols = it["ch"]["qz"] - it["ch"]["qa"]
                if it["first"] or it["last"] or (
                        cost[0] * ACT_COST <= cost[1] * DVE_COST):
                    it["eng"] = "act"
                    cost[0] += cols
                else:
                    it["eng"] = "dve"
                    cost[1] += cols

            head_sb = {}

            def load_head(g):
                if g == 0:
                    kt_parts, qt_parts = [], []
                    koff = 0
                    for i, (a, b) in enumerate(KT0_SLICES):
                        w = (b - a) * CHUNK
                        t = heads.tile([128, w], F16,
                                       tag=f"kt0_{i}", name=f"kt0_{i}")
                        nc.sync.dma_start(
                            out=t,
                            in_=kt0_h[koff:koff + 128 * w].rearrange(
                                "(p x) -> p x", p=128))
                        koff += 128 * w
                        kt_parts.append((a * CHUNK, b * CHUNK, t))
                        if i == 0:
                            q = heads.tile([128, MMF], F16, tag="qt0_0",
                                           name="qt0_0")
                            nc.scalar.dma_start(
                                out=q,
                                in_=qt0_h[0:128 * MMF].rearrange(
                                    "(p x) -> p x", p=128))
                            qt_parts.append((0, MMF, q))
                    qoff = 128 * MMF
                    for i, (a, b) in enumerate(QT0_SLICES[1:], 1):
                        w = (b - a) * MMF
                        q = heads.tile([128, w], F16,
                                       tag=f"qt0_{i}", name=f"qt0_{i}")
                        nc.scalar.dma_start(
                            out=q,
                            in_=qt0_h[qoff:qoff + 128 * w].rearrange(
                                "(p x) -> p x", p=128))
                        qoff += 128 * w
                        qt_parts.append((a * MMF, b * MMF, q))
                else:
                    kt_sb = heads.tile([128, N], F16, tag="kt", bufs=3,
                                       name=f"kt_{g}")
                    nc.sync.dma_start(out=kt_sb, in_=kt_h[g, :, :])
                    qt_sb = heads.tile([128, N], F16, tag="qt", bufs=3,
                                       name=f"qt_{g}")
                    nc.sync.dma_start(out=qt_sb, in_=qt_h[g, :, :])
                    kt_parts = [(0, N, kt_sb)]
                    qt_parts = [(0, N, qt_sb)]
                ve_sb = heads.tile([128, NCHUNK * (D + 1)], BF16, tag="ve",
                                   bufs=3, name=f"ve_{g}")
                nc.gpsimd.dma_start(out=ve_sb, in_=ve_h[g, :, :])
                head_sb[g] = (kt_parts, qt_parts, ve_sb)

            def kt_ap(g, pp, c):
                for a, b, t in head_sb[g][0]:
                    if a <= c * CHUNK < b:
                        off = c * CHUNK - a
                        return t[pp:pp + 64, off:off + CHUNK]
                raise AssertionError

            def qt_ap(g, pp, p, lo, hi):
                qb = p * QP
                for a, b, t in head_sb[g][1]:
                    if a <= qb + lo < b:
                        return t[pp:pp + 64, qb + lo - a:qb + hi - a]
                raise AssertionError

            o_tiles = {}
            n_out = [0]
            out_rings = [nc.sync, nc.gpsimd]

            def end_phase(g, p, f_sb, id_sb):
                qb = p * QP
                NT = QP // 128
                # per-transpose pitch padded to 128 f32 (512B) so no single
                # transpose output crosses a 2KB PSUM bank line (a crossing
                # write corrupts the tail of the transpose)
                t_ps = psum.tile([128, NT * 128], F32R, tag="s", bufs=3,
                                 name=f"t_{g}_{p}")
                for t in range(NT):
                    # D+2 output cols: f32r transpose needs an even
                    # innermost count; the extra identity column is zero
                    nc.tensor.transpose(
                        t_ps[:, t * 128:t * 128 + (D + 2)],
                        f_sb[:, t * 128:(t + 1) * 128],
                        id_sb)
                r_sb = fin.tile([128, NT], F32, tag="r", bufs=4,
                                name=f"r_{g}_{p}")
                t3 = t_ps.bitcast(F32).rearrange("q (t c) -> q t c", c=128)
                nc.vector.reciprocal(r_sb, t3[:, :, D])
                oo_sb = fin.tile([128, NT * D], F32, tag="oo", bufs=4,
                                 name=f"oo_{g}_{p}")
                nc.vector.tensor_tensor(
                    oo_sb.rearrange("q (t d) -> q t d", d=D),
                    t3[:, :, :D], r_sb.to_broadcast([128, NT, D]), mul_op)
                eng = out_rings[n_out[0] % 2]
                n_out[0] += 1
                eng.dma_start(
                    out=o_h[g, qb:qb + QP, :].rearrange("(t p) d -> p t d", p=128),
                    in_=oo_sb.rearrange("p (t d) -> p t d", d=D),
                )

            def emit_pv(it, p_sb, id_sb):
                g, p, ch = it["g"], it["p"], it["ch"]
                if (g, p) not in o_tiles:
                    o_tiles[(g, p)] = psum.tile([D + 1, QP], F32, tag="o",
                                                bufs=1, name=f"o_{g}_{p}")
                o_ps = o_tiles[(g, p)]
                ve_sb = head_sb[g][2]
                c = ch["c"]
                for a in range(0, QP, MMF):
                    if it["first"]:
                        lo, hi = a, a + MMF
                    else:
                        lo, hi = max(a, ch["qa"]), min(a + MMF, ch["qz"])
                    if lo < hi:
                        nc.tensor.matmul(
                            o_ps[:, lo:hi],
                            lhsT=ve_sb[:, c * (D + 1):(c + 1) * (D + 1)],
                            rhs=p_sb[:, lo:hi],
                            start=it["first"], stop=it["last"],
                        )
                if it["last"]:
                    f_sb = fin.tile([D + 1, QP], F32R, tag="f", bufs=3,
                                    name=f"f_{g}_{p}")
                    nc.vector.tensor_copy(f_sb, o_ps.bitcast(F32R))
                    del o_tiles[(g, p)]
                    fin_queue.append([0, (g, p, f_sb)])

            # head 0's tensors gate the first matmuls - their DMAs go first
            load_head(0)
            id_sb = singles.tile([D + 1, D + 2], F32R, tag="ident")
            nc.sync.dma_start(out=id_sb, in_=id_h[:, :])
            me_sb = ms_sb = None
            if use_me:
                me_sb = singles.tile([128, N], BF16, tag="me")
                nc.gpsimd.dma_start(out=me_sb, in_=me_h[:, :])
            if use_ms:
                ms_sb = singles.tile([128, N], BF16, tag="ms")
                nc.gpsimd.dma_start(out=ms_sb, in_=ms_h[:, :])

            pv_queue = []
            fin_queue = []
            for j0 in range(0, len(items), 2):
                pair = items[j0:j0 + 2]
                # stagger head loads: kick off head g+1's DMAs as soon as
                # head g's first pair is in flight
                g_hi = max(it["g"] for it in pair)
                if g_hi + 1 < HPC and g_hi + 1 not in head_sb:
                    load_head(g_hi + 1)
                tiles = []
                # QK matmuls, interleaved across the pair for row-group
                # concurrency (pp = 0 / 64)
                sub = []
                for k, it in enumerate(pair):
                    ch = it["ch"]
                    s_ps = psum.tile([128, QP], F32, tag="s", bufs=3,
                                     name=f"s_{j0}_{k}")
                    tiles.append(s_ps)
                    mms = []
                    for a in range(0, QP, MMF):
                        lo, hi = max(a, ch["qa"]), min(a + MMF, ch["qz"])
                        if lo < hi:
                            mms.append((s_ps, 64 * k, it, lo, hi))
                    sub.append(mms)
                for pr in [x for tup in __import__("itertools")
                           .zip_longest(*sub) for x in tup if x]:
                    s_ps, pp, it, lo, hi = pr
                    g, p = it["g"], it["p"]
                    nc.tensor.matmul(
                        s_ps[:, lo:hi],
                        lhsT=kt_ap(g, pp, it["ch"]["c"]),
                        rhs=qt_ap(g, pp, p, lo, hi),
                        start=True, stop=True,
                        tile_position=(pp, 0),
                    )
                cur = []
                for k, it in enumerate(pair):
                    ch = it["ch"]
                    g, p = it["g"], it["p"]
                    qb = p * QP
                    lo, hi = ch["qa"], ch["qz"]
                    p_sb = pbuf.tile([128, QP], BF16, tag="p", bufs=10,
                                     name=f"p_{j0}_{k}")
                    if it["eng"] == "act":
                        nc.scalar.activation(p_sb[:, lo:hi], tiles[k][:, lo:hi],
                                             exp_f, scale=sm_scale)
                    else:
                        w_sb = pbuf.tile([128, QP], F32R, tag="w", bufs=6,
                                         name=f"w_{j0}_{k}")
                        nc.vector.tensor_scalar(
                            w_sb[:, lo:hi].bitcast(I32), tiles[k][:, lo:hi],
                            a_pool, SCHRAU_B, mul_op, add_op)
                        nc.vector._custom_dve(
                            exp_op, out=p_sb[:, lo:hi],
                            in0=w_sb[:, lo:hi].bitcast(F32),
                            s0=c0, s1=c1, imm2=c2)
                    # p_sb is SBUF, so masks/memsets can run on Pool, which
                    # cannot touch PSUM and is otherwise idle
                    for a, b in (ch["memsets"] if it["first"] else ch["interior"]):
                        nc.gpsimd.memset(p_sb[:, a:b].bitcast(mybir.dt.uint16), 0)
                    for mi, (a, b, m_sb) in enumerate(
                            [(a, b, me_sb) for a, b in ch["mule"]]
                            + [(a, b, ms_sb) for a, b in ch["muls"]]):
                        nc.gpsimd.tensor_mul(p_sb[:, a:b], p_sb[:, a:b],
                                             m_sb[:, qb + a:qb + b])
                    cur.append((it, p_sb))
                if len(pv_queue) >= 2:
                    for it, p_sb in pv_queue.pop(0):
                        emit_pv(it, p_sb, id_sb)
                # deferred end-phase stage2: transposes/normalize run ~2
                # rounds after the pass's CAST so they never block younger
                # QK work in the in-order PE queue
                for e in fin_queue:
                    e[0] += 1
                while fin_queue and fin_queue[0][0] >= 2:
                    _, args = fin_queue.pop(0)
                    end_phase(*args, id_sb)
                pv_queue.append(cur)
            for block in pv_queue:
                for it, p_sb in block:
                    emit_pv(it, p_sb, id_sb)
            for _, args in fin_queue:
                end_phase(*args, id_sb)

    nc.compile()
    return nc


_CACHE = {}


def _get_program(starts, ends, sm_scale, use_me, use_ms):
    key = (starts.tobytes(), ends.tobytes(), float(sm_scale), use_me, use_ms)
    if key not in _CACHE:
        sched = _schedule(starts, ends)
        _CACHE[key] = _build_program(sched, float(sm_scale), use_me, use_ms)
    return _CACHE[key]


def _prep_inputs(q, k, v, starts, ends, use_me, use_ms):
    """Per-core input dicts."""
    qf = np.asarray(q, np.float32).reshape(B * H, N, D)
    kf = np.asarray(k, np.float32).reshape(B * H, N, D)
    vf = np.asarray(v, np.float32).reshape(B * H, N, D)

    # boundary mask strips (shared across heads): column j holds the
    # within-chunk prefix/suffix mask for row_ends[j]/row_starts[j]
    rows = np.arange(128, dtype=np.int64)[:, None]
    me = (rows < (ends[None, :] % CHUNK)).astype(ml_dtypes.bfloat16)
    ms = (rows >= (starts[None, :] % CHUNK)).astype(ml_dtypes.bfloat16)
    ident = np.zeros((D + 1, D + 2), np.float32)
    ident[:, :D + 1] = np.eye(D + 1, dtype=np.float32)

    in_maps = []
    for i in range(NCORES):
        sl = slice(i * HPC, (i + 1) * HPC)
        kt1 = kf[sl].transpose(0, 2, 1).astype(np.float16)   # [HPC, D, N]
        qt1 = qf[sl].transpose(0, 2, 1).astype(np.float16)
        kt = np.ascontiguousarray(np.concatenate([kt1, kt1], axis=1))
        qt = np.ascontiguousarray(np.concatenate([qt1, qt1], axis=1))
        kt0 = np.concatenate(
            [kt[0][:, a * CHUNK:b * CHUNK].ravel() for a, b in KT0_SLICES])
        qt0 = np.concatenate(
            [qt[0][:, a * MMF:b * MMF].ravel() for a, b in QT0_SLICES])
        ve = np.ones([HPC, 128, NCHUNK, D + 1], ml_dtypes.bfloat16)
        ve[:, :, :, :D] = vf[sl].reshape(HPC, NCHUNK, CHUNK, D).transpose(0, 2, 1, 3)
        ve = np.ascontiguousarray(ve.reshape(HPC, 128, NCHUNK * (D + 1)))
        in_maps.append({"kt": kt, "qt": qt, "kt0": kt0, "qt0": qt0,
                        "ve": ve, "me": me, "ms": ms, "ident": ident})
    return in_maps


def _run(inputs, trace=False):
    q, k, v = inputs["q"], inputs["k"], inputs["v"]
    sm_scale = float(np.asarray(inputs["sm_scale"]))
    starts_raw = np.asarray(inputs["row_starts"], np.int64)
    ends_raw = np.asarray(inputs["row_ends"], np.int64)
    starts = np.clip(starts_raw, 0, N)
    ends = np.clip(ends_raw, 0, N)

    use_ms = bool((starts % CHUNK).any())
    use_me = bool(((ends % CHUNK) * (ends > starts)).any())

    nc = _get_program(starts, ends, sm_scale, use_me, use_ms)
    in_maps = _prep_inputs(q, k, v, starts, ends, use_me, use_ms)
    res = run_bass_kernel_spmd(nc, in_maps, list(range(NCORES)), trace=trace)

    out = np.empty([B * H, N, D], np.float32)
    for i in range(NCORES):
        out[i * HPC:(i + 1) * HPC] = res.results[i]["o"]
    out = out.reshape(B, H, N, D)

    empty = ends <= starts
    if empty.any():
        mean_v = np.asarray(v, np.float32).mean(axis=2)          # [B, H, D]
        out[:, :, empty, :] = mean_v[:, :, None, :]
    return out, res.exec_time_ns


def kernel(**inputs) -> np.ndarray:
    out, _ = _run(inputs, trace=False)
    return out
